# revision 9
# baseline (speedup 1.0000x reference)
"""Trainium2 Bass kernel for CTC loss (keras ctc_batch_cost port).

Strategy
--------
Pure data parallel across 8 NeuronCores: 32 batch elements per core.

Per core:
  Phase 1 (gather):  for each (b, ti) load y_pred[b, ti*128:(ti+1)*128, :]
     into SBUF, GPSIMD ap_gather the 129 extended-label columns
     (blank,l1,blank,...,blank; padded to 144), add EPS on ACT, store to a
     DRAM staging buffer pbuf[b, t, j].
  Phase 2 (DP):      linear-space CTC forward recursion over T=512 steps on
     the Vector engine, batch on partitions (32), states on the free dim
     (S=129).  alpha is renormalized every 8 steps by K/max (K=2^58 centers
     the state spread inside fp32's exponent range; the raw max is logged
     for the final answer).  Per step:
        u    = A[s] + A[s-1]                     (shifted-AP add)
        u   += A[s-2] * skip    (odd states only)
        A'   = (u * rinv?) * p_t                 (scalar_tensor_tensor)
  Phase 3 (finalize): ll = ln(alpha[S-1]+alpha[S-2]) + sum(ln(m_i/K));
     output -ll.

Numerics: matches the fp32 log-space reference to ~1e-7 rel (validated in
numpy); the K-centered renorm is required — plain max-renorm loses mid-state
trajectories that dip ~e^-90 below the max and later feed the final states.
"""
import numpy as np
import concourse.bacc as bacc
import concourse.tile as tile
from concourse import mybir
from concourse.bass_utils import run_bass_kernel_spmd

F32 = mybir.dt.float32
I16 = mybir.dt.int16
ALU = mybir.AluOpType
ACTF = mybir.ActivationFunctionType
AXL = mybir.AxisListType

B, T, C, L = 256, 512, 512, 64
S = 2 * L + 1            # 129
NCORES = 8
BC = B // NCORES         # 32 batch elements per core
BLANK = C - 1            # 511
EPS = 1e-7
NIDX = 144               # gather width (multiple of 16 >= S)
NW = NIDX // 16          # 9 wrapped index columns
NWP = 10                 # padded stride per batch element (20B, 4-byte aligned;
                         # GPSIMD ap_gather requires 4B-aligned idx offsets)
TI = 128                 # time rows per phase-1 tile
NTI = T // TI            # 4
CH = 64                  # DP chunk length (time steps per SBUF p-tile)
NCH = T // CH            # 8
RN = 8                   # renorm period
NRN = (T - 1) // RN      # 63 renorms (t = 8,16,...,504)
K_SCALE = float(2.0 ** 58)
K_INV = float(2.0 ** -58)

_NC_CACHE = None


def _build():
    nc = bacc.Bacc("TRN2", target_bir_lowering=False, debug=False)
    d_yp = nc.dram_tensor("yp", [BC, T, C], F32, kind="ExternalInput")
    d_gidx = nc.dram_tensor("gidx", [128, BC * NWP], I16, kind="ExternalInput")
    d_skipm = nc.dram_tensor("skipm", [BC, L], F32, kind="ExternalInput")
    d_out = nc.dram_tensor("out", [BC, 1], F32, kind="ExternalOutput")

    with tile.TileContext(nc) as tc, \
         tc.tile_pool(name="const", bufs=1) as constp, \
         tc.tile_pool(name="ypp", bufs=3) as ypp, \
         tc.tile_pool(name="gp", bufs=3) as gp, \
         tc.tile_pool(name="pcp", bufs=2) as pcp, \
         tc.tile_pool(name="dpp", bufs=1) as dpp, \
         tc.tile_pool(name="dramp", bufs=1, space="DRAM") as dramp:

        t_gidx = constp.tile([128, BC * NWP], I16, tag="gidx")
        nc.sync.dma_start(t_gidx[:], d_gidx[:])
        t_skipm = constp.tile([BC, L], F32, tag="skipm")
        nc.sync.dma_start(t_skipm[:], d_skipm[:])
        t_eps = constp.tile([128, 1], F32, tag="eps")
        nc.vector.memset(t_eps[:], EPS)

        pbufs = [
            dramp.tile([BC, TI, NIDX], F32, tag=f"pbuf{ti}", name=f"pbuf{ti}")
            for ti in range(NTI)
        ]

        # ---------------- Phase 1: gather ----------------
        for ti in range(NTI):
            for b in range(BC):
                t_yp = ypp.tile([TI, C], F32, tag="yp")
                nc.sync.dma_start(t_yp[:], d_yp[b, ti * TI:(ti + 1) * TI, :])
                t_g = gp.tile([TI, NIDX], F32, tag="g")
                nc.gpsimd.ap_gather(
                    t_g[:], t_yp[:], t_gidx[:, b * NWP:b * NWP + NW],
                    channels=128, num_elems=C, d=1, num_idxs=NIDX,
                )
                nc.scalar.add(t_g[:], t_g[:], t_eps[:, 0:1])
                nc.sync.dma_start(pbufs[ti][b], t_g[:])

        # ---------------- Phase 2: DP ----------------
        t_A = dpp.tile([BC, S + 2], F32, tag="A")       # state s at col 2+s
        t_u = dpp.tile([BC, S], F32, tag="u")
        t_w = dpp.tile([BC, L], F32, tag="w")
        t_rbuf = dpp.tile([BC, NRN], F32, tag="rbuf")   # raw renorm maxes
        t_rcp = dpp.tile([BC, 1], F32, tag="rcp")
        t_rinv = dpp.tile([BC, 1], F32, tag="rinv")

        nc.vector.memset(t_A[:], 0.0)

        for c in range(NCH):
            t_p = pcp.tile([BC, CH, NIDX], F32, tag="pch")
            nc.sync.dma_start(
                t_p[:], pbufs[c // 2][:, (c % 2) * CH:(c % 2 + 1) * CH, :])
            for tl in range(CH):
                tg = c * CH + tl
                if tg == 0:
                    nc.vector.tensor_copy(t_A[:, 2:4], t_p[:, 0, 0:2])
                    continue
                p_t = t_p[:, tl, 0:S]
                nc.vector.tensor_add(t_u[:], t_A[:, 2:2 + S], t_A[:, 1:1 + S])
                nc.vector.tensor_mul(t_w[:], t_A[:, 1:S:2], t_skipm[:])
                nc.vector.tensor_add(t_u[:, 1:S:2], t_u[:, 1:S:2], t_w[:])
                if tg % RN == 0:
                    ri = tg // RN - 1
                    nc.vector.tensor_reduce(
                        t_rbuf[:, ri:ri + 1], t_u[:], AXL.X, ALU.max)
                    nc.vector.reciprocal(t_rcp[:], t_rbuf[:, ri:ri + 1])
                    nc.vector.tensor_scalar_mul(t_rinv[:], t_rcp[:], K_SCALE)
                    nc.vector.scalar_tensor_tensor(
                        t_A[:, 2:2 + S], t_u[:], t_rinv[:, 0:1], p_t,
                        op0=ALU.mult, op1=ALU.mult)
                else:
                    nc.vector.tensor_mul(t_A[:, 2:2 + S], t_u[:], p_t)

        # ---------------- Phase 3: finalize ----------------
        t_f0 = dpp.tile([BC, 1], F32, tag="f0")
        nc.vector.tensor_add(t_f0[:], t_A[:, S:S + 1], t_A[:, S + 1:S + 2])
        t_fl = dpp.tile([BC, 1], F32, tag="fl")
        nc.scalar.activation(t_fl[:], t_f0[:], ACTF.Ln)
        t_rl = dpp.tile([BC, NRN], F32, tag="rl")
        # ln(m * 2^-58) = ln m - 58 ln 2, via activation's free scale
        nc.scalar.activation(t_rl[:], t_rbuf[:], ACTF.Ln, scale=K_INV)
        t_rs = dpp.tile([BC, 1], F32, tag="rs")
        nc.vector.tensor_reduce(t_rs[:], t_rl[:], AXL.X, ALU.add)
        t_res = dpp.tile([BC, 1], F32, tag="res")
        nc.vector.tensor_scalar(
            t_res[:], t_fl[:], t_rs[:, 0:1], -1.0, op0=ALU.add, op1=ALU.mult)
        nc.sync.dma_start(d_out[:], t_res[:])

    nc.compile()
    return nc


def _host_prep(y_true, y_pred):
    """Build per-core input maps (tiny y_true-derived index/mask tensors)."""
    y_true = np.asarray(y_true)
    y_pred = np.asarray(y_pred, dtype=np.float32)
    assert y_true.shape == (B, L), y_true.shape
    assert y_pred.shape == (B, T, C), y_pred.shape

    idx = np.zeros((B, NIDX), np.int16)
    idx[:, 0:S:2] = BLANK
    idx[:, 1:S:2] = y_true.astype(np.int16)
    w = idx.reshape(B, NW, 16)                      # [B, scol, k]

    skipm = np.zeros((B, L), np.float32)
    skipm[:, 1:] = (y_true[:, 1:] != y_true[:, :-1]).astype(np.float32)

    in_maps = []
    for cc in range(NCORES):
        sl = slice(cc * BC, (cc + 1) * BC)
        wc = w[sl]                                  # [BC, NW, 16]
        gidx9 = np.tile(wc.transpose(2, 0, 1), (8, 1, 1))   # [128, BC, NW]
        gidx = np.zeros((128, BC, NWP), np.int16)
        gidx[:, :, :NW] = gidx9
        gidx = gidx.reshape(128, BC * NWP)
        in_maps.append({
            "yp": np.ascontiguousarray(y_pred[sl]),
            "gidx": np.ascontiguousarray(gidx.astype(np.int16)),
            "skipm": np.ascontiguousarray(skipm[sl]),
        })
    return in_maps


def kernel(y_true, y_pred):
    global _NC_CACHE
    in_maps = _host_prep(y_true, y_pred)
    if _NC_CACHE is None:
        _NC_CACHE = _build()
    res = run_bass_kernel_spmd(_NC_CACHE, in_maps, core_ids=list(range(NCORES)))
    out = np.concatenate([res.results[cc]["out"] for cc in range(NCORES)], axis=0)
    return np.ascontiguousarray(out.astype(np.float32))


# revision 10
# speedup vs baseline: 1.0226x; 1.0226x over previous
"""Trainium2 Bass kernel for CTC loss (keras ctc_batch_cost port).

Strategy
--------
Pure data parallel across 8 NeuronCores: 32 batch elements per core.

Per core:
  Phase 1 (gather):  for each (b, ti) load y_pred[b, ti*128:(ti+1)*128, :]
     into SBUF, GPSIMD ap_gather the 129 extended-label columns
     (blank,l1,blank,...,blank; padded to 144), add EPS on ACT, store to a
     DRAM staging buffer pbuf[b, t, j].
  Phase 2 (DP):      linear-space CTC forward recursion over T=512 steps on
     the Vector engine, batch on partitions (32), states on the free dim
     (S=129).  alpha is renormalized every 8 steps by K/max (K=2^58 centers
     the state spread inside fp32's exponent range; the raw max is logged
     for the final answer).  Per step:
        u    = A[s] + A[s-1]                     (shifted-AP add)
        u   += A[s-2] * skip    (odd states only)
        A'   = (u * rinv?) * p_t                 (scalar_tensor_tensor)
  Phase 3 (finalize): ll = ln(alpha[S-1]+alpha[S-2]) + sum(ln(m_i/K));
     output -ll.

Numerics: matches the fp32 log-space reference to ~1e-7 rel (validated in
numpy); the K-centered renorm is required — plain max-renorm loses mid-state
trajectories that dip ~e^-90 below the max and later feed the final states.
"""
import numpy as np
import concourse.bacc as bacc
import concourse.tile as tile
from concourse import mybir
from concourse.bass_utils import run_bass_kernel_spmd

F32 = mybir.dt.float32
I16 = mybir.dt.int16
ALU = mybir.AluOpType
ACTF = mybir.ActivationFunctionType
AXL = mybir.AxisListType

B, T, C, L = 256, 512, 512, 64
S = 2 * L + 1            # 129
NCORES = 8
BC = B // NCORES         # 32 batch elements per core
BLANK = C - 1            # 511
EPS = 1e-7
NIDX = 144               # gather width (multiple of 16 >= S)
NW = NIDX // 16          # 9 wrapped index columns
NWP = 10                 # padded stride per batch element (20B, 4-byte aligned;
                         # GPSIMD ap_gather requires 4B-aligned idx offsets)
TI = 128                 # time rows per phase-1 tile
NTI = T // TI            # 4
CH = 64                  # DP chunk length (time steps per SBUF p-tile)
NCH = T // CH            # 8
RN = 8                   # renorm period
NRN = (T - 1) // RN      # 63 renorms (t = 8,16,...,504)
K_SCALE = float(2.0 ** 58)
K_INV = float(2.0 ** -58)

_NC_CACHE = None


def _build():
    nc = bacc.Bacc("TRN2", target_bir_lowering=False, debug=False)
    d_yp = nc.dram_tensor("yp", [BC, T, C], F32, kind="ExternalInput")
    d_gidx = nc.dram_tensor("gidx", [128, BC * NWP], I16, kind="ExternalInput")
    d_skipm = nc.dram_tensor("skipm", [BC, L], F32, kind="ExternalInput")
    d_out = nc.dram_tensor("out", [BC, 1], F32, kind="ExternalOutput")

    with tile.TileContext(nc) as tc, \
         tc.tile_pool(name="const", bufs=1) as constp, \
         tc.tile_pool(name="ypp", bufs=3) as ypp, \
         tc.tile_pool(name="gp", bufs=3) as gp, \
         tc.tile_pool(name="pcp", bufs=2) as pcp, \
         tc.tile_pool(name="dpp", bufs=1) as dpp, \
         tc.tile_pool(name="dramp", bufs=1, space="DRAM") as dramp:

        t_gidx = constp.tile([128, BC * NWP], I16, tag="gidx")
        nc.sync.dma_start(t_gidx[:], d_gidx[:])
        t_skipm = constp.tile([BC, L], F32, tag="skipm")
        nc.sync.dma_start(t_skipm[:], d_skipm[:])
        t_eps = constp.tile([128, 1], F32, tag="eps")
        nc.vector.memset(t_eps[:], EPS)

        pbufs = [
            dramp.tile([BC, TI, NIDX], F32, tag=f"pbuf{ti}", name=f"pbuf{ti}")
            for ti in range(NTI)
        ]

        # ---------------- Phase 1: gather ----------------
        for ti in range(NTI):
            for b in range(BC):
                t_yp = ypp.tile([TI, C], F32, tag="yp")
                nc.sync.dma_start(t_yp[:], d_yp[b, ti * TI:(ti + 1) * TI, :])
                t_g = gp.tile([TI, NIDX], F32, tag="g")
                nc.gpsimd.ap_gather(
                    t_g[:], t_yp[:], t_gidx[:, b * NWP:b * NWP + NW],
                    channels=128, num_elems=C, d=1, num_idxs=NIDX,
                )
                nc.scalar.add(t_g[:], t_g[:], t_eps[:, 0:1])
                nc.sync.dma_start(pbufs[ti][b], t_g[:])

        # ---------------- Phase 2: DP ----------------
        t_A = dpp.tile([BC, S + 2], F32, tag="A")       # state s at col 2+s
        t_u = dpp.tile([BC, S], F32, tag="u")
        t_w = dpp.tile([BC, L], F32, tag="w")
        t_rbuf = dpp.tile([BC, NRN], F32, tag="rbuf")   # raw renorm maxes
        t_rcp = dpp.tile([BC, 1], F32, tag="rcp")
        t_rinv = dpp.tile([BC, 1], F32, tag="rinv")

        nc.vector.memset(t_A[:], 0.0)

        for c in range(NCH):
            t_p = pcp.tile([BC, CH, NIDX], F32, tag="pch")
            nc.sync.dma_start(
                t_p[:], pbufs[c // 2][:, (c % 2) * CH:(c % 2 + 1) * CH, :])
            for tl in range(CH):
                tg = c * CH + tl
                if tg == 0:
                    nc.vector.tensor_copy(t_A[:, 2:4], t_p[:, 0, 0:2])
                    continue
                p_t = t_p[:, tl, 0:S]
                # Each op is split into two independent half-state-range ops
                # and interleaved so no op directly follows its producer —
                # hides part of the DVE self-sem turnaround (~2% end to end).
                HS = 65  # states 0..64 | 65..128
                nc.vector.tensor_add(t_u[:, 0:HS], t_A[:, 2:2 + HS], t_A[:, 1:1 + HS])
                nc.vector.tensor_add(t_u[:, HS:S], t_A[:, 2 + HS:2 + S], t_A[:, 1 + HS:1 + S])
                nc.vector.tensor_mul(t_w[:, 0:32], t_A[:, 1:64:2], t_skipm[:, 0:32])
                nc.vector.tensor_mul(t_w[:, 32:64], t_A[:, 65:128:2], t_skipm[:, 32:64])
                nc.vector.tensor_add(t_u[:, 1:64:2], t_u[:, 1:64:2], t_w[:, 0:32])
                nc.vector.tensor_add(t_u[:, 65:128:2], t_u[:, 65:128:2], t_w[:, 32:64])
                if tg % RN == 0:
                    ri = tg // RN - 1
                    nc.vector.tensor_reduce(
                        t_rbuf[:, ri:ri + 1], t_u[:], AXL.X, ALU.max)
                    nc.vector.reciprocal(t_rcp[:], t_rbuf[:, ri:ri + 1])
                    nc.vector.tensor_scalar_mul(t_rinv[:], t_rcp[:], K_SCALE)
                    nc.vector.scalar_tensor_tensor(
                        t_A[:, 2:2 + HS], t_u[:, 0:HS], t_rinv[:, 0:1], p_t[:, 0:HS],
                        op0=ALU.mult, op1=ALU.mult)
                    nc.vector.scalar_tensor_tensor(
                        t_A[:, 2 + HS:2 + S], t_u[:, HS:S], t_rinv[:, 0:1], p_t[:, HS:S],
                        op0=ALU.mult, op1=ALU.mult)
                else:
                    nc.vector.tensor_mul(t_A[:, 2:2 + HS], t_u[:, 0:HS], p_t[:, 0:HS])
                    nc.vector.tensor_mul(t_A[:, 2 + HS:2 + S], t_u[:, HS:S], p_t[:, HS:S])

        # ---------------- Phase 3: finalize ----------------
        t_f0 = dpp.tile([BC, 1], F32, tag="f0")
        nc.vector.tensor_add(t_f0[:], t_A[:, S:S + 1], t_A[:, S + 1:S + 2])
        t_fl = dpp.tile([BC, 1], F32, tag="fl")
        nc.scalar.activation(t_fl[:], t_f0[:], ACTF.Ln)
        t_rl = dpp.tile([BC, NRN], F32, tag="rl")
        # ln(m * 2^-58) = ln m - 58 ln 2, via activation's free scale
        nc.scalar.activation(t_rl[:], t_rbuf[:], ACTF.Ln, scale=K_INV)
        t_rs = dpp.tile([BC, 1], F32, tag="rs")
        nc.vector.tensor_reduce(t_rs[:], t_rl[:], AXL.X, ALU.add)
        t_res = dpp.tile([BC, 1], F32, tag="res")
        nc.vector.tensor_scalar(
            t_res[:], t_fl[:], t_rs[:, 0:1], -1.0, op0=ALU.add, op1=ALU.mult)
        nc.sync.dma_start(d_out[:], t_res[:])

    nc.compile()
    return nc


def _host_prep(y_true, y_pred):
    """Build per-core input maps (tiny y_true-derived index/mask tensors)."""
    y_true = np.asarray(y_true)
    y_pred = np.asarray(y_pred, dtype=np.float32)
    assert y_true.shape == (B, L), y_true.shape
    assert y_pred.shape == (B, T, C), y_pred.shape

    idx = np.zeros((B, NIDX), np.int16)
    idx[:, 0:S:2] = BLANK
    idx[:, 1:S:2] = y_true.astype(np.int16)
    w = idx.reshape(B, NW, 16)                      # [B, scol, k]

    skipm = np.zeros((B, L), np.float32)
    skipm[:, 1:] = (y_true[:, 1:] != y_true[:, :-1]).astype(np.float32)

    in_maps = []
    for cc in range(NCORES):
        sl = slice(cc * BC, (cc + 1) * BC)
        wc = w[sl]                                  # [BC, NW, 16]
        gidx9 = np.tile(wc.transpose(2, 0, 1), (8, 1, 1))   # [128, BC, NW]
        gidx = np.zeros((128, BC, NWP), np.int16)
        gidx[:, :, :NW] = gidx9
        gidx = gidx.reshape(128, BC * NWP)
        in_maps.append({
            "yp": np.ascontiguousarray(y_pred[sl]),
            "gidx": np.ascontiguousarray(gidx.astype(np.int16)),
            "skipm": np.ascontiguousarray(skipm[sl]),
        })
    return in_maps


def kernel(y_true, y_pred):
    global _NC_CACHE
    in_maps = _host_prep(y_true, y_pred)
    if _NC_CACHE is None:
        _NC_CACHE = _build()
    res = run_bass_kernel_spmd(_NC_CACHE, in_maps, core_ids=list(range(NCORES)))
    out = np.concatenate([res.results[cc]["out"] for cc in range(NCORES)], axis=0)
    return np.ascontiguousarray(out.astype(np.float32))
